# revision 1
# baseline (speedup 1.0000x reference)
"""AliasFreeActivation Trainium2 kernel (v21).

out = crop10(down2(leaky_relu(up4(x + bias)) * sqrt2))   [4,256,236,236]

Per (batch,channel) image (1024 images, 128 per core), with A the up4
matrix [128,512] and D the down2 matrix [512,256] (both banded):

  s1  v1[w,ho]  = sum_h xb[h,w] A[h,ho]              1 MM N=512
  s2  Y[ho,wo]  = sum_w v1[w,ho] (sqrt2*A)[w,wo]     4 MM N=512
  act L = prelu(Y, 0.2)   one fused ScalarE pass per PSUM pair
          (= sqrt2*leaky_relu(up4(xb)); sqrt2 folded into s2's matrix;
           ACT Prelu honors alpha — Lrelu does NOT, its slope is 0.01)
  s3  z[wo,hd]  = sum_ho L[ho,wo] D[ho,hd]          16 MM banded N<=70
  s4  oT[wd,hd] = sum_wo D[wo,wd] z[wo,hd]           5 MM N=236
      (constant D stationary -> no per-image weight loads scale with the
       data; output lands transposed and is fixed on the host)

Why this shape: the PE's serial resources are the LDWEIGHTS stream
(~0.8 ns/col; 26 loads/image here) and the moving stream; the PSUM
evacuation engines (ScalarE+VectorE, ~1 ns/elem, the only two that can
read PSUM) are co-critical.  The bias-add is folded on the host into x
(free), the fine-grid activation is a single-instruction Prelu on
ScalarE over two-bank [128,2,512] APs, VectorE carries the V1/z/out
casts (the second output group's copy is split along the FREE dim so
ScalarE and VectorE land balanced at ~2.5us/image each — PSUM reads
must keep partition base 0, so a partition split is illegal), and I/O
is fp16 both ways with one input and one output DMA per image.  s4 accumulates both 128-row output groups in ONE PSUM bank
(start=True only on the bank's first MM; has_written handles the rest).

Partial-partition matmul outputs (M<128) accumulate incorrectly on this
stack — all matmuls here write full-128-partition outputs.
"""
import numpy as np

UP, DOWN, MARGIN, NEG_SLOPE = 4, 2, 10, 0.2
SQRT2 = 1.4142135623730951
H = W = 128
OUT = 236
NCORES = 8
NIMG = 128

# down-matrix window per 128-row K-chunk: D[s,o] nonzero for o in [64k-3, 64k+66]
DWIN = [(0, 67), (61, 131), (125, 195), (189, 256)]
# s4 blocks (chunk k, out-group g) with g0 = wd 10..137, g1 = wd 138..265
S4MM = [(0, 0), (1, 0), (2, 0), (2, 1), (3, 1)]

CM_A = 0
CM_A2 = 512
CM_DW = 1024          # 4 windows, 70 cols apart
CM_DH = 1304          # 5 dense [128,128] blocks
CM_SA = 1944          # column sums of A, replicated across partitions
CM_COLS = 2456
VERSION = 77          # bump on every kernel change: cache-key nonce

_cache = {}


def _build_nc(nimg=NIMG):
    import concourse.bacc as bacc
    import concourse.bass as bass
    import concourse.tile as tile
    from concourse import mybir

    F32 = mybir.dt.float32
    F16 = mybir.dt.float16
    AF = mybir.ActivationFunctionType
    ALU = mybir.AluOpType

    nc = bacc.Bacc("TRN2", target_bir_lowering=False)
    x_d = nc.dram_tensor("x", [nimg, H, W], F16, kind="ExternalInput")
    c_d = nc.dram_tensor("cm", [128, CM_COLS], F16, kind="ExternalInput")
    nc.dram_tensor("nonce", [1, VERSION], F16, kind="ExternalInput")
    o_d = nc.dram_tensor("out", [nimg, 2, 128, OUT], F16, kind="ExternalOutput")

    with tile.TileContext(nc) as tc:
        with (
            tc.tile_pool(name="const", bufs=1) as const,
            tc.tile_pool(name="xin", bufs=10) as xin,
            tc.tile_pool(name="v1p", bufs=3) as v1p,
            tc.tile_pool(name="yp", bufs=3) as yp,
            tc.tile_pool(name="zp", bufs=3) as zp,
            tc.tile_pool(name="ofp", bufs=6) as ofp,
            tc.tile_pool(name="p1", bufs=1, space="PSUM") as p1p,
            tc.tile_pool(name="p2", bufs=2, space="PSUM") as p2p,
            tc.tile_pool(name="p34", bufs=1, space="PSUM") as p34p,
            tc.tile_pool(name="pt", bufs=1, space="PSUM") as ptp,
        ):
            cm = const.tile([128, CM_COLS], F16)
            nc.sync.dma_start(out=cm, in_=c_d[:])
            A_sb = cm[:, CM_A:CM_A + 512]
            A2_sb = cm[:, CM_A2:CM_A2 + 512]

            def D_sb(k):
                o0, o1 = DWIN[k]
                return cm[:, CM_DW + 70 * k: CM_DW + 70 * k + (o1 - o0)]

            def Dh_sb(j):
                return cm[:, CM_DH + 128 * j: CM_DH + 128 * (j + 1)]

            # warm PE's clock on the const DMA lane
            pwarm = p2p.tile([128, 2, 512], F32, name="p2")
            nc.tensor.matmul(out=pwarm[:32, 0, :256], lhsT=cm[:, :32],
                             rhs=cm[:, :256], start=True, stop=True)

            def s1_mm(i):
                # s1: up vertical (bias folds in during evacuation)
                X = xin.tile([128, W], F16)
                nc.sync.dma_start(out=X, in_=x_d[i])
                P1 = p1p.tile([128, 512], F32)
                nc.tensor.matmul(out=P1, lhsT=X, rhs=A_sb,
                                 start=True, stop=True)
                return P1

            def s1_evac(i, P1):
                V1 = v1p.tile([128, 512], F16)
                nc.vector.tensor_copy(out=V1, in_=P1)
                return V1

            def s4_mm(ip, Z):
                # s4: down horizontal with D stationary -> transposed out
                PT = ptp.tile([128, 2, OUT], F32)
                for j, (k, g) in enumerate(S4MM):
                    nc.tensor.matmul(out=PT[:, g, :], lhsT=Dh_sb(j),
                                     rhs=Z[:, k, :],
                                     start=(j == 0), stop=(j == len(S4MM) - 1))
                return (ip, PT)

            def of_evac(ip, PT):
                # deferred one iteration: deps are long done, so these
                # never block the engine FIFOs
                OF = ofp.tile([128, 2, OUT], F16)
                nc.vector.tensor_copy(out=OF[:, 0, :], in_=PT[:, 0, :])
                # split g1 along the free dim (partition base must stay 0
                # for PSUM reads) to balance ACT vs DVE busy time
                nc.scalar.copy(out=OF[0:OUT - 128, 1, 0:225],
                               in_=PT[0:OUT - 128, 1, 0:225])
                nc.vector.tensor_copy(out=OF[0:OUT - 128, 1, 225:OUT],
                                      in_=PT[0:OUT - 128, 1, 225:OUT])
                nc.sync.dma_start(
                    out=bass.AP(tensor=o_d[:].tensor,
                                offset=ip * 2 * 128 * OUT,
                                ap=[[OUT, 128], [128 * OUT, 2], [1, OUT]]),
                    in_=OF)

            for i in range(nimg):
                V1 = s1_evac(i, s1_mm(i))

                # s2 + fused leaky-relu evacuation (fine grid)
                Y = yp.tile([128, 4, 512], F16)
                for pr in range(2):
                    P2 = p2p.tile([128, 2, 512], F32, name="p2")
                    for h in range(2):
                        m = 2 * pr + h
                        nc.tensor.matmul(out=P2[:, h, :],
                                         lhsT=V1[:, 128 * m:128 * (m + 1)],
                                         rhs=A2_sb, start=True, stop=True)
                    nc.scalar.activation(out=Y[:, 2 * pr:2 * pr + 2, :],
                                         in_=P2, func=AF.Prelu,
                                         bias=0.0, scale=1.0, alpha=NEG_SLOPE)

                if i > 0:
                    of_evac(*PTlag)

                # s3: down vertical (banded), all four wo-chunks in one
                # 2-bank PSUM tile, single evacuation
                P34 = p34p.tile([128, 4, 256], F32)
                for m in range(4):
                    for k in range(4):
                        o0, o1 = DWIN[k]
                        nc.tensor.matmul(
                            out=P34[:, m, o0:o1],
                            lhsT=Y[:, k, 128 * m:128 * (m + 1)],
                            rhs=D_sb(k), start=(k == 0), stop=(k == 3))
                Z = zp.tile([128, 4, OUT], F16)
                nc.vector.tensor_copy(out=Z,
                                      in_=P34[:, :, MARGIN:MARGIN + OUT])

                PTlag = s4_mm(i, Z)

            of_evac(*PTlag)

    nc.finalize()
    return nc


def _filter_matrices(up_filter, down_filter):
    fu = np.asarray(up_filter, dtype=np.float64)
    fd = np.asarray(down_filter, dtype=np.float64)
    i = np.arange(128)[:, None]
    o = np.arange(512)[None, :]
    t = 10 + o - 4 * i
    A = np.where((t >= 0) & (t < 24), fu[np.clip(t, 0, 23)], 0.0)
    s = np.arange(512)[:, None]
    o2 = np.arange(256)[None, :]
    t2 = 6 + 2 * o2 - s
    D = np.where((t2 >= 0) & (t2 < 12), fd[np.clip(t2, 0, 11)], 0.0)
    return A, D


def _pack_consts(up_filter, down_filter):
    A, D = _filter_matrices(up_filter, down_filter)
    cm = np.zeros((128, CM_COLS), dtype=np.float16)
    cm[:, CM_A:CM_A + 512] = A.astype(np.float16)
    cm[:, CM_A2:CM_A2 + 512] = (A * SQRT2).astype(np.float16)
    for k, (o0, o1) in enumerate(DWIN):
        cm[:, CM_DW + 70 * k: CM_DW + 70 * k + (o1 - o0)] = \
            D[128 * k:128 * (k + 1), o0:o1].astype(np.float16)
    Dpad = np.concatenate([D, np.zeros((512, 10))], axis=1)
    for j, (k, g) in enumerate(S4MM):
        c0 = MARGIN + 128 * g
        cm[:, CM_DH + 128 * j: CM_DH + 128 * (j + 1)] = \
            Dpad[128 * k:128 * (k + 1), c0:c0 + 128].astype(np.float16)
    cm[:, CM_SA:CM_SA + 512] = np.tile(A.sum(axis=0, keepdims=True),
                                       (128, 1)).astype(np.float16)
    return cm


def _run(x, bias, up_filter, down_filter, trace=False):
    import os
    # the NEFF compile cache is keyed on the HLO wrapper, which does not
    # include this kernel's BIR (it rides in backend_config) — stale-NEFF
    # collisions are possible, so always recompile
    os.environ["NEURON_FORCE_RECOMPILE"] = "1"
    from concourse.bass_utils import run_bass_kernel_spmd

    if "nc" not in _cache:
        _cache["nc"] = _build_nc()
    nc = _cache["nc"]

    cm = _pack_consts(up_filter, down_filter)
    xb = np.asarray(x, dtype=np.float32) + \
        np.asarray(bias, dtype=np.float32)[None, :, None, None]
    xf = np.ascontiguousarray(xb.astype(np.float16)
                              .reshape(NCORES * NIMG, H, W))

    in_maps = []
    for c in range(NCORES):
        in_maps.append({
            "x": xf[NIMG * c: NIMG * (c + 1)],
            "cm": cm,
            "nonce": np.zeros((1, VERSION), dtype=np.float16),
        })
    res = run_bass_kernel_spmd(nc, in_maps, core_ids=list(range(NCORES)),
                               trace=trace)
    out = np.concatenate([res.results[c]["out"][None] for c in range(NCORES)], 0)
    out = out.reshape(NCORES * NIMG, 2, 128, OUT)
    out = np.concatenate([out[:, 0, :, :], out[:, 1, 0:OUT - 128, :]], axis=1)
    out = out.reshape(4, 256, OUT, OUT)
    # device produced [wd, hd]; reference wants [hd, wd]
    out = np.ascontiguousarray(out.swapaxes(2, 3)).astype(np.float32)
    return out, res


def kernel(x, bias, up_filter, down_filter):
    out, _ = _run(x, bias, up_filter, down_filter, trace=False)
    return out


def kernel_traced(x, bias, up_filter, down_filter):
    return _run(x, bias, up_filter, down_filter, trace=True)

